# revision 5
# baseline (speedup 1.0000x reference)
"""Fused multi-head-attention-block kernel for 8 Trainium2 NeuronCores.

Reference computation (B=4, S=2048, D=1024):
    qp  = q @ Wq + bq
    k   = qp @ Wk
    v   = qp @ Wv + bv
    qk  = einsum('bsd,btd->bst', qp, k) * (D//16)**-0.25 + mask
    out = softmax(qk) @ v @ Wo + bo

Sharding: each core owns one (batch, half-of-queries) pair.  Each core
computes the full-batch projections (qpT, kT, v) from the host-transposed
qT, then attention + output projection for its 1024 query rows only.

All activations are kept transposed ([dim, seq]) so that every matmul's
contraction dim lands on SBUF partitions with zero on-chip transposes:
    qpT = Wq^T @ qT          kT = Wk^T @ qpT        v = qpT^T @ Wv
    qkT[t,s] = kT^T@qpT      wvT = v^T @ E          out = wvT^T @ Wo
Softmax runs over the PARTITION dim of qkT: exp on ScalarE (scale fused,
no max-subtraction -- logits are bounded and exp(-1e9) -> 0 exactly on the
ACT LUT), multiplicative mask tiles M = exp(mask) from the host (handles
causal and arbitrary additive masks uniformly), denominator via a
ones-vector matmul accumulated across key tiles, division folded into the
PSUM eviction of the PV matmul.

Causal masks additionally enable block skipping: a host-side query
permutation gives every core the same {8,16}-key-tile loop structure
(SPMD needs one program), cutting attention matmuls by 25%.

Matmuls run in float32r (TF32-like, 4x fp32 throughput, ~2e-4 rel err).
"""

import os
import sys
from contextlib import ExitStack

for _p in ("/opt/trn_rl_repo",):
    if _p not in sys.path and os.path.isdir(_p):
        sys.path.append(_p)

import numpy as np

import concourse.bass as bass
import concourse.mybir as mybir
import concourse.tile as tile
from concourse import bacc
from concourse.bass_utils import run_bass_kernel_spmd

B, S, D, N_HEAD = 4, 2048, 1024, 16
P = 128                      # SBUF partitions
NCORES = 8
SQC = 512                    # query-chunk width (free dim of every matmul)
DK = D // P                  # 8 contraction tiles
NKT = S // P                 # 16 key tiles
SCALE = float((D // N_HEAD) ** (-0.25))
F32 = mybir.dt.float32
F32R = mybir.dt.float32r

_PROGRAM_CACHE = {}


def _build_program(kt_depths):
    """Emit the SPMD Bass program.  kt_depths = (ktA, ktB): number of
    128-row key tiles processed for query chunk A (positions 0:512) and
    chunk B (positions 1536:2048)."""
    nc = bacc.Bacc("TRN2", target_bir_lowering=False, debug=False,
                   num_devices=NCORES)

    qT = nc.dram_tensor("qT", [D, S], F32R, kind="ExternalInput").ap()
    M = nc.dram_tensor("M", [sum(kt_depths), P, SQC], F32,
                       kind="ExternalInput").ap()
    wq = nc.dram_tensor("Wq", [D, D], F32R, kind="ExternalInput").ap()
    wk = nc.dram_tensor("Wk", [D, D], F32R, kind="ExternalInput").ap()
    wv = nc.dram_tensor("Wv", [D, D], F32R, kind="ExternalInput").ap()
    wo = nc.dram_tensor("Wo", [D, D], F32R, kind="ExternalInput").ap()
    bq = nc.dram_tensor("bq", [D], F32, kind="ExternalInput").ap()
    bv = nc.dram_tensor("bv", [D], F32, kind="ExternalInput").ap()
    bo = nc.dram_tensor("bo", [D], F32, kind="ExternalInput").ap()
    out = nc.dram_tensor("out", [2 * SQC, D], F32, kind="ExternalOutput").ap()
    # kT spill: [kt, dk, 128, 128] so one DMA restores a full [128, 1024] tile
    kTd = nc.dram_tensor("kTd", [NKT, DK, P, P], F32R).ap()

    # query-chunk start positions in the permuted coordinate space
    chunk_pos = (0, 3 * SQC)

    with tile.TileContext(nc) as tc, ExitStack() as stack:
        const = stack.enter_context(tc.tile_pool(name="const", bufs=1))
        ones_f = const.tile([P, 1], F32)
        nc.gpsimd.memset(ones_f[:], 1.0)
        ones_r = const.tile([P, 1], F32R)
        nc.vector.tensor_copy(ones_r[:], ones_f[:])
        bq_sb = const.tile([P, DK], F32)
        nc.sync.dma_start(out=bq_sb[:], in_=bq.rearrange("(a p) -> p a", p=P))
        bv_row = const.tile([1, D], F32)
        nc.sync.dma_start(out=bv_row[:], in_=bv.unsqueeze(0))
        bo_row = const.tile([1, D], F32)
        nc.sync.dma_start(out=bo_row[:], in_=bo.unsqueeze(0))
        bv_bc = const.tile([P, D], F32)
        nc.gpsimd.partition_broadcast(bv_bc[:], bv_row[:])
        bo_bc = const.tile([P, D], F32)
        nc.gpsimd.partition_broadcast(bo_bc[:], bo_row[:])

        # persistent activation storage
        qpa_p = stack.enter_context(tc.tile_pool(name="qpa", bufs=1))
        qpb_p = stack.enter_context(tc.tile_pool(name="qpb", bufs=1))
        v_p = stack.enter_context(tc.tile_pool(name="vp", bufs=1))
        qpa = [qpa_p.tile([P, SQC], F32R, name=f"qpa{d}") for d in range(DK)]
        qpb = [qpb_p.tile([P, SQC], F32R, name=f"qpb{d}") for d in range(DK)]
        vt = [v_p.tile([P, D], F32R, name=f"v{t}") for t in range(NKT)]

        psum_acc = stack.enter_context(tc.tile_pool(name="ps_acc", bufs=2, space="PSUM"))
        psum_den = stack.enter_context(tc.tile_pool(name="ps_den", bufs=1, space="PSUM"))
        psum_pv = stack.enter_context(tc.tile_pool(name="ps_pv", bufs=1, space="PSUM"))

        # ---- Phases A-C: projections ------------------------------------
        with tc.tile_pool(name="wpool", bufs=9) as wp, \
             tc.tile_pool(name="qtpool", bufs=8) as qtp, \
             tc.tile_pool(name="qpo", bufs=1) as qpo_p, \
             tc.tile_pool(name="kev", bufs=2) as kev:

            qpo = [qpo_p.tile([P, 2 * SQC], F32R, name=f"qpo{d}")
                   for d in range(DK)]

            w_q = [wp.tile([P, D], F32R, name=f"wq{d}", tag="w")
                   for d in range(DK)]
            for d in range(DK):
                nc.sync.dma_start(out=w_q[d][:], in_=wq[d * P:(d + 1) * P, :])

            # Phase A: qpT[do, s] = sum_dk Wq[dk,do]^T qT[dk, s]  (+ bq)
            for sc in range(S // SQC):
                qts = [qtp.tile([P, SQC], F32R, name=f"qt{sc}_{dk}", tag="qt")
                       for dk in range(DK)]
                for dk in range(DK):
                    nc.sync.dma_start(
                        out=qts[dk][:],
                        in_=qT[dk * P:(dk + 1) * P, sc * SQC:(sc + 1) * SQC])
                for do in range(DK):
                    acc = psum_acc.tile([P, SQC], F32, tag="acc")
                    for dk in range(DK):
                        nc.tensor.matmul(
                            out=acc[:], lhsT=w_q[dk][:, do * P:(do + 1) * P],
                            rhs=qts[dk][:], start=(dk == 0), stop=(dk == DK - 1))
                    if sc == 0:
                        dst = qpa[do][:]
                    elif sc == 3:
                        dst = qpb[do][:]
                    else:
                        dst = qpo[do][:, (sc - 1) * SQC:sc * SQC]
                    nc.vector.tensor_tensor(
                        dst, acc[:],
                        bq_sb[:, do:do + 1].to_broadcast([P, SQC]),
                        mybir.AluOpType.add)

            def qp_col(dk, sc):
                if sc == 0:
                    return qpa[dk][:]
                if sc == 3:
                    return qpb[dk][:]
                return qpo[dk][:, (sc - 1) * SQC:sc * SQC]

            # Phase B: kT = Wk^T @ qpT  -> DRAM spill (tile layout)
            w_k = [wp.tile([P, D], F32R, name=f"wk{d}", tag="w")
                   for d in range(DK)]
            for d in range(DK):
                nc.sync.dma_start(out=w_k[d][:], in_=wk[d * P:(d + 1) * P, :])
            for sc in range(S // SQC):
                for do in range(DK):
                    acc = psum_acc.tile([P, SQC], F32, tag="acc")
                    for dk in range(DK):
                        nc.tensor.matmul(
                            out=acc[:], lhsT=w_k[dk][:, do * P:(do + 1) * P],
                            rhs=qp_col(dk, sc), start=(dk == 0),
                            stop=(dk == DK - 1))
                    kt_sb = kev.tile([P, SQC], F32R, name=f"kev{sc}_{do}",
                                     tag="kev")
                    nc.vector.tensor_copy(kt_sb[:], acc[:])
                    # [128, 512] -> kTd[4*sc:4*sc+4, do, :, :]
                    nc.sync.dma_start(
                        out=kTd[4 * sc:4 * (sc + 1), do].transpose([1, 0, 2]),
                        in_=kt_sb[:].rearrange("p (a f) -> p a f", a=4))

            # Phase C: v[s, dv] = sum_dk qpT[dk, s]^T Wv[dk, dv]  (+ bv)
            w_v = [wp.tile([P, D], F32R, name=f"wv{d}", tag="w")
                   for d in range(DK)]
            for d in range(DK):
                nc.sync.dma_start(out=w_v[d][:], in_=wv[d * P:(d + 1) * P, :])
            for st in range(NKT):
                sc, off = st // 4, (st % 4) * P
                for nch in range(2):
                    acc = psum_acc.tile([P, SQC], F32, tag="acc")
                    for dk in range(DK):
                        nc.tensor.matmul(
                            out=acc[:],
                            lhsT=qp_col(dk, sc)[:, off:off + P],
                            rhs=w_v[dk][:, nch * SQC:(nch + 1) * SQC],
                            start=(dk == 0), stop=(dk == DK - 1))
                    nc.vector.tensor_tensor(
                        vt[st][:, nch * SQC:(nch + 1) * SQC], acc[:],
                        bv_bc[:, nch * SQC:(nch + 1) * SQC],
                        mybir.AluOpType.add)

        # ---- Phase D: attention + output projection ---------------------
        with tc.tile_pool(name="kts", bufs=2) as ktsp, \
             tc.tile_pool(name="mp", bufs=2) as mp, \
             tc.tile_pool(name="ep", bufs=2) as ep, \
             tc.tile_pool(name="erp", bufs=NKT) as erp, \
             tc.tile_pool(name="rp", bufs=2) as rp, \
             tc.tile_pool(name="wvt", bufs=1) as wvtp, \
             tc.tile_pool(name="wop", bufs=8) as wop, \
             tc.tile_pool(name="osb", bufs=2) as osb:

            m_off = 0
            for ci, nkt_c in enumerate(kt_depths):
                cpos = chunk_pos[ci]
                qp_mine = qpa if ci == 0 else qpb

                # pass 1: E_r[kt] = exp(SCALE * qkT) * M
                ers = []
                for kt in range(nkt_c):
                    kt_sb = ktsp.tile([P, D], F32R, name=f"kts{ci}_{kt}",
                                      tag="kts")
                    nc.sync.dma_start(out=kt_sb[:].rearrange(
                        "p (a f) -> p a f", a=DK),
                        in_=kTd[kt].transpose([1, 0, 2]))
                    acc = psum_acc.tile([P, SQC], F32, tag="acc")
                    for dk in range(DK):
                        nc.tensor.matmul(
                            out=acc[:], lhsT=kt_sb[:, dk * P:(dk + 1) * P],
                            rhs=qp_mine[dk][:], start=(dk == 0),
                            stop=(dk == DK - 1))
                    e_t = ep.tile([P, SQC], F32, name=f"e{ci}_{kt}", tag="e")
                    nc.scalar.activation(e_t[:], acc[:],
                                         mybir.ActivationFunctionType.Exp,
                                         bias=0.0, scale=SCALE)
                    m_t = mp.tile([P, SQC], F32, name=f"m{ci}_{kt}", tag="m")
                    nc.sync.dma_start(out=m_t[:], in_=M[m_off + kt])
                    er = erp.tile([P, SQC], F32R, name=f"er{ci}_{kt}",
                                  tag="er")
                    nc.vector.tensor_tensor(er[:], e_t[:], m_t[:],
                                            mybir.AluOpType.mult)
                    ers.append(er)
                m_off += nkt_c

                # denominator: den[s] = sum_t E_r[t, s]
                den = psum_den.tile([1, SQC], F32, tag="den")
                for kt in range(nkt_c):
                    nc.tensor.matmul(out=den[:], lhsT=ones_r[:],
                                     rhs=ers[kt][:], start=(kt == 0),
                                     stop=(kt == nkt_c - 1))
                recip = rp.tile([1, SQC], F32, name=f"recip{ci}", tag="recip")
                nc.vector.reciprocal(recip[:], den[:])
                recip_bc = rp.tile([P, SQC], F32, name=f"recipbc{ci}",
                                   tag="recipbc")
                nc.gpsimd.partition_broadcast(recip_bc[:], recip[:])

                # pass 2: wvT[dv, s] = (sum_t v[t, dv] E_r[t, s]) / den[s]
                wvts = [wvtp.tile([P, SQC], F32R, name=f"wvt{ci}_{d}",
                                  tag=f"wvt{d}") for d in range(DK)]
                for dh in range(2):
                    pvs = []
                    for dc in range(4):
                        pv = psum_pv.tile([P, SQC], F32, tag=f"pv{dc}")
                        dv = dh * 4 + dc
                        for kt in range(nkt_c):
                            nc.tensor.matmul(
                                out=pv[:],
                                lhsT=vt[kt][:, dv * P:(dv + 1) * P],
                                rhs=ers[kt][:], start=(kt == 0),
                                stop=(kt == nkt_c - 1))
                        pvs.append(pv)
                    for dc in range(4):
                        nc.vector.tensor_tensor(wvts[dh * 4 + dc][:], pvs[dc][:],
                                                recip_bc[:],
                                                mybir.AluOpType.mult)

                # pass 3: out rows = wvT^T @ Wo + bo
                for nch in range(2):
                    wos = [wop.tile([P, SQC], F32R, name=f"wo{ci}_{nch}_{d}",
                                    tag="wo") for d in range(DK)]
                    for dk in range(DK):
                        nc.sync.dma_start(
                            out=wos[dk][:],
                            in_=wo[dk * P:(dk + 1) * P,
                                   nch * SQC:(nch + 1) * SQC])
                    for st in range(4):
                        acc = psum_acc.tile([P, SQC], F32, tag="acc")
                        for dk in range(DK):
                            nc.tensor.matmul(
                                out=acc[:],
                                lhsT=wvts[dk][:, st * P:(st + 1) * P],
                                rhs=wos[dk][:], start=(dk == 0),
                                stop=(dk == DK - 1))
                        o_sb = osb.tile([P, SQC], F32, name=f"o{ci}_{nch}_{st}",
                                        tag="osb")
                        nc.vector.tensor_tensor(
                            o_sb[:], acc[:],
                            bo_bc[:, nch * SQC:(nch + 1) * SQC],
                            mybir.AluOpType.add)
                        nc.sync.dma_start(
                            out=out[ci * SQC + st * P:ci * SQC + (st + 1) * P,
                                    nch * SQC:(nch + 1) * SQC],
                            in_=o_sb[:])

    nc.compile()
    return nc


def _get_program(kt_depths):
    if kt_depths not in _PROGRAM_CACHE:
        _PROGRAM_CACHE[kt_depths] = _build_program(kt_depths)
    return _PROGRAM_CACHE[kt_depths]


def _perms():
    """Permuted query order per half.  Core (b, h) query chunk A sits at
    positions 0:512, chunk B at positions 1536:2048."""
    idx = np.arange(S)
    perm0 = idx.copy()                       # h=0: identity
    perm1 = np.concatenate([idx[512:1024], idx[0:512],
                            idx[1536:2048], idx[1024:1536]])
    return perm0, perm1


def _is_causal(mask):
    i = np.arange(S)
    tri = np.where(i[:, None] >= i[None, :], np.float32(0.0),
                   np.float32(-1e9))
    return np.array_equal(mask, tri)


def kernel(q, mask, Wq, bq, Wk, Wv, bv, Wo, bo):
    q = np.asarray(q, dtype=np.float32)
    mask = np.asarray(mask, dtype=np.float32)
    causal = _is_causal(mask)
    kt_depths = (8, 16) if causal else (16, 16)
    nc = _get_program(kt_depths)

    perm0, perm1 = _perms()
    perms = (perm0, perm1)
    emask = np.exp(mask.astype(np.float64)).astype(np.float32)

    # mask tiles per half-variant: M[c_kt, i, j] = exp(mask)[q_pos j, k_pos i]
    m_variants = []
    for h in range(2):
        pm = perms[h]
        tiles = []
        for ci, nkt_c in enumerate(kt_depths):
            cpos = (0, 3 * SQC)[ci]
            qrows = pm[cpos:cpos + SQC]
            for kt in range(nkt_c):
                krows = pm[kt * P:(kt + 1) * P]
                tiles.append(emask[np.ix_(qrows, krows)].T)
        m_variants.append(np.ascontiguousarray(np.stack(tiles)))

    in_maps = []
    for c in range(NCORES):
        b, h = c // 2, c % 2
        qTp = np.ascontiguousarray(q[b][perms[h]].T)
        in_maps.append({
            "qT": qTp, "M": m_variants[h],
            "Wq": Wq, "Wk": Wk, "Wv": Wv, "Wo": Wo,
            "bq": np.asarray(bq, np.float32),
            "bv": np.asarray(bv, np.float32),
            "bo": np.asarray(bo, np.float32),
        })

    res = run_bass_kernel_spmd(nc, in_maps, core_ids=list(range(NCORES)))

    out = np.empty((B, S, D), dtype=np.float32)
    for c in range(NCORES):
        b, h = c // 2, c % 2
        pm = perms[h]
        co = res.results[c]["out"]
        out[b, pm[0:SQC]] = co[0:SQC]
        out[b, pm[3 * SQC:4 * SQC]] = co[SQC:2 * SQC]
    return out
